# revision 4
# baseline (speedup 1.0000x reference)
"""Trainium2 Bass kernel for nn_KGEModel_57741540327562 (HousE-style KGE scoring).

Strategy (v5):
  - Data-parallel over the batch dim: 8 cores x 32 batch rows each.
  - entity_embedding replicated per core as an fp8(e4m3) table whose rows are
    de-interleaved to [x0(256) | x1(256)] and pre-scaled by S=4096 (so fp8
    only sees well-normalized values; the 1/S is folded into the epilogue).
  - Small relation/type tables folded on the host into per-(b,d) QR
    coefficients (exact float64 math); coefficients pre-broadcast to
    [bc, 128, 1280] bf16 and streamed via HWDGE.
  - Per 2-batch-row chunk: 8 indirect SWDGE gathers (fp8 -> bf16 cast during
    DMA), 5 wide DVE ops for the linear part, ACT Square + Sqrt, a short bf16
    halving tree + fp32 tensor_reduce for the d-sum. TensorE/PSUM unused.

score[b,n] = GAMMA - (1/S) * sum_d sqrt( (Sa0-t00x0'-t01x1')^2 + (Sa1-t11x1')^2 )
where x' = S*x, [t00 t01; 0 t11] = Q^T @ (diag(rw) @ M_tail) per (b,d),
a = Q^T (rw * head).
"""
import sys

sys.path.insert(0, "/opt/trn_rl_repo")

import numpy as np
import ml_dtypes

NE, NR, NT = 200000, 1000, 571
D, HD = 256, 2
HOUSE_NUM, HOUSD = 6, 1
GAMMA, THRED, RTHRED = 10.0, 0.5, 0.8
B, NEG, NCORES = 256, 512, 8
BC = B // NCORES     # batch rows per core
NT4 = NEG // 128     # 128-row gather tiles per batch row
CB = 2               # batch rows per compute chunk
NCH = BC // CB       # chunks per core
S = 4096.0           # fp8 pre-scale
BF16 = ml_dtypes.bfloat16
F8 = ml_dtypes.float8_e4m3


def _l2norm(x, axis=-1):
    n = np.sqrt(np.sum(x * x, axis=axis, keepdims=True))
    return x / np.maximum(n, 1e-12)


def _reflect(x, r, k=0.0):
    c = np.sum(r * x, axis=-1, keepdims=True)
    return x - (2.0 + k) * c * r


def precompute(inputs):
    """Host-side prep: fold small tables into per-(b,d) QR coefficients.

    Returns (table_f8 [NE,512] fp8 scaled by S, cof [B,1280] float32 with
    the a-halves pre-scaled by S): cof row = [-t00 | -t01 | -t11 | Sa0 | Sa1].
    """
    f8t = np.float64
    ent = np.asarray(inputs["entity_embedding"], f8t)         # [NE,D,2]
    rel_emb = np.asarray(inputs["relation_embedding"], f8t)   # [NR,D,12]
    htm = np.asarray(inputs["head_type_mat"], f8t)            # [NT,D,2]
    ttm = np.asarray(inputs["tail_type_mat"], f8t)
    r1_dir = np.asarray(inputs["r1_dir_head"], f8t)           # [NT,1,1]
    r2_dir = np.asarray(inputs["r2_dir_tail"], f8t)
    r1_sc = np.asarray(inputs["r1_scale_head"], f8t)          # [NT,D,1]
    r2_sc = np.asarray(inputs["r2_scale_tail"], f8t)
    k_dir_h = np.asarray(inputs["k_dir_head"], f8t)           # [NR,1,1]
    k_dir_t = np.asarray(inputs["k_dir_tail"], f8t)
    k_sc_h = np.asarray(inputs["k_scale_head"], f8t)          # [NR,D,1]
    k_sc_t = np.asarray(inputs["k_scale_tail"], f8t)
    rw = np.asarray(inputs["relation_weight"], f8t)           # [NR,D,2]
    htv = np.asarray(inputs["head_type_vec"])                 # [NE] int
    hp = np.asarray(inputs["head_part"])                      # [B,3] int

    r = _l2norm(rel_emb.reshape(NR, D, HOUSE_NUM, HD))        # [NR,D,6,2]
    r1n = _l2norm(htm.reshape(NT, D, 1, HD)).reshape(NT, D, HD)
    r2n = _l2norm(ttm.reshape(NT, D, 1, HD)).reshape(NT, D, HD)
    k_head = np.minimum(k_dir_h * np.abs(k_sc_h), THRED)      # [NR,D,1]
    k_tail = np.minimum(k_dir_t * np.abs(k_sc_t), THRED)
    r1_head = np.minimum(r1_dir * np.abs(r1_sc), RTHRED)      # [NT,D,1]
    r2_tail = np.minimum(r2_dir * np.abs(r2_sc), RTHRED)

    h_id, rel_id, t_id = hp[:, 0], hp[:, 1], hp[:, 2]
    htyp = htv[h_id]
    ttyp = htv[t_id]

    # ---- head transform (exact chain on [B,D,2]) ----
    head = ent[h_id]                                          # [B,D,2]
    head = _reflect(head, r1n[htyp], r1_head[htyp])
    rel = r[rel_id]                                           # [B,D,6,2]
    head = _reflect(head, rel[:, :, 0, :], k_head[rel_id])
    for i in range(HOUSD, HOUSE_NUM - HOUSD):
        head = _reflect(head, rel[:, :, i, :])

    # ---- tail transform matrix M[b,d] (2x2): x -> A2 @ A1 @ x ----
    def _refl_mat(rv, k):
        I = np.eye(2)[None, None]
        outer = rv[..., :, None] * rv[..., None, :]
        return I - (2.0 + k)[..., None] * outer

    A1 = _refl_mat(r2n[ttyp], r2_tail[ttyp][:, :, 0:1])
    A2 = _refl_mat(rel[:, :, HOUSE_NUM - 1, :], k_tail[rel_id])
    M = A2 @ A1                                               # [B,D,2,2]

    rwg = rw[rel_id]                                          # [B,D,2]
    Mt = rwg[..., :, None] * M                                # diag(rw) @ M
    a = rwg * head                                            # [B,D,2]

    # ---- Givens QR: Mt = Q T, T upper-triangular ----
    u0, u1 = Mt[..., 0, 0], Mt[..., 0, 1]
    v0, v1 = Mt[..., 1, 0], Mt[..., 1, 1]
    rho = np.sqrt(u0 * u0 + v0 * v0)
    rho_s = np.maximum(rho, 1e-30)
    c, s = u0 / rho_s, v0 / rho_s
    t00 = rho
    t01 = c * u1 + s * v1
    t11 = -s * u1 + c * v1
    a0p = c * a[..., 0] + s * a[..., 1]
    a1p = -s * a[..., 0] + c * a[..., 1]

    cof = np.concatenate([-t00, -t01, -t11, S * a0p, S * a1p],
                         axis=1).astype(np.float32)           # [B,1280]

    # ---- fp8 table: de-interleave to [x0 | x1], scale by S ----
    e32 = np.asarray(inputs["entity_embedding"], np.float32)
    table = (np.concatenate([e32[:, :, 0], e32[:, :, 1]], axis=1)
             * np.float32(S)).astype(F8)                      # [NE,512]
    return table, cof


def emulate(inputs):
    """Numpy emulation of the device numerics for validation."""
    table, cof = precompute(inputs)
    tp = np.asarray(inputs["tail_part"])
    cof = cof.astype(BF16).astype(np.float32)                 # [B,1280]
    bf = lambda z: z.astype(BF16).astype(np.float32)
    rows = table[tp].astype(np.float32)                       # cast fp8->bf16
    x0, x1 = rows[:, :, :256], rows[:, :, 256:]
    T00, T01 = cof[:, None, 0:256], cof[:, None, 256:512]
    T11 = cof[:, None, 512:768]
    A0, A1 = cof[:, None, 768:1024], cof[:, None, 1024:1280]
    w0 = bf(x0 * T00)
    w1 = bf(x1 * T01)
    d0 = bf(bf(w0 + w1) + A0)
    d1 = bf(bf(x1 * T11) + A1)
    e = bf(bf(d0 * d0) + bf(d1 * d1))
    sq = bf(np.sqrt(e))
    r = sq
    for _ in range(3):
        h = r.shape[-1] // 2
        r = bf(r[..., :h] + r[..., h:])
    red = np.sum(r.astype(np.float32), axis=-1)
    return (GAMMA - red / S).astype(np.float32)


# ----------------------------------------------------------------------------
# Device program
# ----------------------------------------------------------------------------
def build_nc5(ne=NE, bc=BC, nt4=NT4, cb=CB):
    import concourse.bacc as bacc
    import concourse.mybir as mybir
    from concourse.bass import IndirectOffsetOnAxis
    from concourse.tile import TileContext

    dt = mybir.dt
    nc = bacc.Bacc("TRN2", target_bir_lowering=False, debug=False,
                   num_devices=NCORES)
    tab = nc.dram_tensor("tab", [ne, 2 * D], dt.float8e4,
                         kind="ExternalInput").ap()
    idx = nc.dram_tensor("idx", [128, bc * nt4], dt.int32,
                         kind="ExternalInput").ap()
    cofb = nc.dram_tensor("cofb", [bc, 128, 5 * D], dt.bfloat16,
                          kind="ExternalInput").ap()
    out = nc.dram_tensor("scores", [128, bc * nt4], dt.float32,
                         kind="ExternalOutput").ap()

    mult, add = mybir.AluOpType.mult, mybir.AluOpType.add
    SQRT = mybir.ActivationFunctionType.Sqrt
    SQ = mybir.ActivationFunctionType.Square
    nch = bc // cb
    U = cb * nt4                     # gather tiles per chunk

    def bc4(ap_slice, shape):
        # [128, cb, W] -> [128, cb, nt4, W] with step-0 over nt4
        w = ap_slice.shape[-1]
        return ap_slice.rearrange("p b (o w) -> p b o w", o=1).to_broadcast(
            shape)

    with TileContext(nc) as tc:
        with (
            tc.tile_pool(name="pidx", bufs=1) as pidx,
            tc.tile_pool(name="pcof", bufs=3) as pcof,
            tc.tile_pool(name="px", bufs=3) as px,
            tc.tile_pool(name="pw", bufs=2) as pw,
            tc.tile_pool(name="pe", bufs=2) as pe,
            tc.tile_pool(name="psc", bufs=1) as psc,
        ):
            ixt = pidx.tile([128, bc * nt4], dt.int32, tag="ix")
            nc.sync.dma_start(out=ixt[:, 0:U], in_=idx[:, 0:U])
            if U < bc * nt4:
                nc.sync.dma_start(out=ixt[:, U:], in_=idx[:, U:])
            score = psc.tile([128, bc * nt4], dt.float32, tag="sc")

            for ch in range(nch):
                u0 = ch * U
                ct = pcof.tile([128, cb, 5 * D], dt.bfloat16, tag="cof")
                nc.sync.dma_start(
                    out=ct[:],
                    in_=cofb[ch * cb:(ch + 1) * cb, :, :].rearrange(
                        "b p w -> p b w"))
                X = px.tile([128, U, 2 * D], dt.bfloat16, tag="x")
                for j in range(U):
                    nc.gpsimd.indirect_dma_start(
                        out=X[:, j, :], out_offset=None, in_=tab[:],
                        in_offset=IndirectOffsetOnAxis(
                            ap=ixt[:, u0 + j:u0 + j + 1], axis=0))
                Xv = X[:].rearrange("p (b o) w -> p b o w", b=cb)
                sh2 = [128, cb, nt4, 2 * D]
                sh1 = [128, cb, nt4, D]
                # W = [x0|x1] * [-t00|-t01]
                W = pw.tile(sh2, dt.bfloat16, tag="w")
                nc.vector.tensor_tensor(out=W[:], in0=Xv,
                                        in1=bc4(ct[:, :, 0:512], sh2),
                                        op=mult)
                # D = [d0 | d1] (pre-constant)
                Dt = pw.tile(sh2, dt.bfloat16, tag="d")
                nc.vector.tensor_tensor(out=Dt[:, :, :, 0:256],
                                        in0=W[:, :, :, 0:256],
                                        in1=W[:, :, :, 256:512], op=add)
                nc.vector.tensor_tensor(out=Dt[:, :, :, 256:512],
                                        in0=Xv[:, :, :, 256:512],
                                        in1=bc4(ct[:, :, 512:768], sh1),
                                        op=mult)
                # D += [S*a0 | S*a1]
                nc.vector.tensor_tensor(out=Dt[:], in0=Dt[:],
                                        in1=bc4(ct[:, :, 768:1280], sh2),
                                        op=add)
                SQt = pw.tile(sh2, dt.bfloat16, tag="sq")
                nc.scalar.activation(SQt[:], Dt[:], SQ)
                E = pe.tile(sh1, dt.bfloat16, tag="e")
                nc.vector.tensor_tensor(out=E[:], in0=SQt[:, :, :, 0:256],
                                        in1=SQt[:, :, :, 256:512], op=add)
                Sq = pe.tile(sh1, dt.bfloat16, tag="s")
                nc.scalar.activation(Sq[:], E[:], SQRT)
                R1 = pe.tile([128, cb, nt4, D // 2], dt.bfloat16, tag="r1")
                nc.vector.tensor_tensor(out=R1[:], in0=Sq[:, :, :, 0:128],
                                        in1=Sq[:, :, :, 128:256], op=add)
                R2 = pe.tile([128, cb, nt4, D // 4], dt.bfloat16, tag="r2")
                nc.vector.tensor_tensor(out=R2[:], in0=R1[:, :, :, 0:64],
                                        in1=R1[:, :, :, 64:128], op=add)
                R3 = pe.tile([128, cb, nt4, D // 8], dt.bfloat16, tag="r3")
                nc.vector.tensor_tensor(out=R3[:], in0=R2[:, :, :, 0:32],
                                        in1=R2[:, :, :, 32:64], op=add)
                nc.vector.tensor_reduce(
                    out=score[:, u0:u0 + U].rearrange("p (u o) -> p u o",
                                                      o=1),
                    in_=R3[:], axis=mybir.AxisListType.X,
                    op=add)

            fin = psc.tile([128, bc * nt4], dt.float32, tag="fin")
            nc.vector.tensor_scalar(out=fin[:], in0=score[:],
                                    scalar1=-1.0 / S, scalar2=GAMMA,
                                    op0=mult, op1=add)
            nc.sync.dma_start(out=out[:, :], in_=fin[:])
    nc.compile()
    return nc


def _in_maps(inputs):
    table, cof = precompute(inputs)
    tp = np.asarray(inputs["tail_part"]).astype(np.int32)     # [B,NEG]
    cof_bc = np.ascontiguousarray(
        np.broadcast_to(cof.astype(BF16)[:, None, :], (B, 128, 5 * D)))
    maps = []
    for c in range(NCORES):
        bs = slice(c * BC, (c + 1) * BC)
        ix = tp[bs].reshape(BC, NT4, 128).transpose(2, 0, 1).reshape(
            128, BC * NT4).copy()
        maps.append({
            "tab": table,
            "idx": np.ascontiguousarray(ix),
            "cofb": np.ascontiguousarray(cof_bc[bs]),
        })
    return maps


def unscramble(arr, bc=BC, nt4=NT4):
    """[128, bc*nt4] device layout -> [bc, NEG]: scores[b, t*128+p]."""
    return np.ascontiguousarray(
        arr.reshape(128, bc, nt4).transpose(1, 2, 0).reshape(bc, nt4 * 128))


def kernel(**inputs) -> np.ndarray:
    from concourse import bass_utils

    nc = build_nc5()
    res = bass_utils.run_bass_kernel_spmd(
        nc, _in_maps(inputs), core_ids=list(range(NCORES)))
    outs = [unscramble(np.asarray(r["scores"])) for r in res.results]
    return np.concatenate(outs, axis=0).astype(np.float32)


def timed_run(inputs):
    """Traced run for test.py; returns max-core exec time in ns."""
    from concourse import bass_utils

    nc = build_nc5()
    res = bass_utils.run_bass_kernel_spmd(
        nc, _in_maps(inputs), core_ids=list(range(NCORES)), trace=True)
    return res.exec_time_ns


if __name__ == "__main__":
    # quick numpy validation against the reference
    sys.path.insert(0, "/root/problem")
    import os
    os.environ.setdefault("JAX_PLATFORMS", "cpu")
    import reference
    inputs = {k: np.asarray(v) for k, v in reference.setup_inputs().items()}
    exp = np.asarray(reference.reference(**reference.setup_inputs()))
    got = emulate(inputs)
    err = np.abs(got - exp) / np.maximum(np.abs(exp), 1e-6)
    print("emulate rel err: max", err.max(), "mean", err.mean())


# revision 8
# speedup vs baseline: 1.0180x; 1.0180x over previous
"""Trainium2 Bass kernel for nn_KGEModel_57741540327562 (HousE-style KGE scoring).

Strategy (v5):
  - Data-parallel over the batch dim: 8 cores x 32 batch rows each.
  - entity_embedding replicated per core as an fp8(e4m3) table whose rows are
    de-interleaved to [x0(256) | x1(256)] and pre-scaled by S=4096 (so fp8
    only sees well-normalized values; the 1/S is folded into the epilogue).
  - Small relation/type tables folded on the host into per-(b,d) QR
    coefficients (exact float64 math); coefficients pre-broadcast to
    [bc, 128, 1280] bf16 and streamed via HWDGE.
  - Per 2-batch-row chunk: 8 indirect SWDGE gathers (fp8 -> bf16 cast during
    DMA), 5 wide DVE ops for the linear part, ACT Square + Sqrt, a short bf16
    halving tree + fp32 tensor_reduce for the d-sum. TensorE/PSUM unused.

score[b,n] = GAMMA - (1/S) * sum_d sqrt( (Sa0-t00x0'-t01x1')^2 + (Sa1-t11x1')^2 )
where x' = S*x, [t00 t01; 0 t11] = Q^T @ (diag(rw) @ M_tail) per (b,d),
a = Q^T (rw * head).
"""
import sys

sys.path.insert(0, "/opt/trn_rl_repo")

import numpy as np
import ml_dtypes

NE, NR, NT = 200000, 1000, 571
D, HD = 256, 2
HOUSE_NUM, HOUSD = 6, 1
GAMMA, THRED, RTHRED = 10.0, 0.5, 0.8
B, NEG, NCORES = 256, 512, 8
BC = B // NCORES     # batch rows per core
NT4 = NEG // 128     # 128-row gather tiles per batch row
CB = 2               # batch rows per compute chunk
NCH = BC // CB       # chunks per core
S = 4096.0           # fp8 pre-scale
BF16 = ml_dtypes.bfloat16
F8 = ml_dtypes.float8_e4m3


def _l2norm(x, axis=-1):
    n = np.sqrt(np.sum(x * x, axis=axis, keepdims=True))
    return x / np.maximum(n, 1e-12)


def _reflect(x, r, k=0.0):
    c = np.sum(r * x, axis=-1, keepdims=True)
    return x - (2.0 + k) * c * r


def precompute(inputs):
    """Host-side prep: fold small tables into per-(b,d) QR coefficients.

    Returns (table_f8 [NE,512] fp8 scaled by S, cof [B,1280] float32 with
    the a-halves pre-scaled by S): cof row = [-t00 | -t01 | -t11 | Sa0 | Sa1].
    """
    f8t = np.float64
    ent = np.asarray(inputs["entity_embedding"], f8t)         # [NE,D,2]
    rel_emb = np.asarray(inputs["relation_embedding"], f8t)   # [NR,D,12]
    htm = np.asarray(inputs["head_type_mat"], f8t)            # [NT,D,2]
    ttm = np.asarray(inputs["tail_type_mat"], f8t)
    r1_dir = np.asarray(inputs["r1_dir_head"], f8t)           # [NT,1,1]
    r2_dir = np.asarray(inputs["r2_dir_tail"], f8t)
    r1_sc = np.asarray(inputs["r1_scale_head"], f8t)          # [NT,D,1]
    r2_sc = np.asarray(inputs["r2_scale_tail"], f8t)
    k_dir_h = np.asarray(inputs["k_dir_head"], f8t)           # [NR,1,1]
    k_dir_t = np.asarray(inputs["k_dir_tail"], f8t)
    k_sc_h = np.asarray(inputs["k_scale_head"], f8t)          # [NR,D,1]
    k_sc_t = np.asarray(inputs["k_scale_tail"], f8t)
    rw = np.asarray(inputs["relation_weight"], f8t)           # [NR,D,2]
    htv = np.asarray(inputs["head_type_vec"])                 # [NE] int
    hp = np.asarray(inputs["head_part"])                      # [B,3] int

    r = _l2norm(rel_emb.reshape(NR, D, HOUSE_NUM, HD))        # [NR,D,6,2]
    r1n = _l2norm(htm.reshape(NT, D, 1, HD)).reshape(NT, D, HD)
    r2n = _l2norm(ttm.reshape(NT, D, 1, HD)).reshape(NT, D, HD)
    k_head = np.minimum(k_dir_h * np.abs(k_sc_h), THRED)      # [NR,D,1]
    k_tail = np.minimum(k_dir_t * np.abs(k_sc_t), THRED)
    r1_head = np.minimum(r1_dir * np.abs(r1_sc), RTHRED)      # [NT,D,1]
    r2_tail = np.minimum(r2_dir * np.abs(r2_sc), RTHRED)

    h_id, rel_id, t_id = hp[:, 0], hp[:, 1], hp[:, 2]
    htyp = htv[h_id]
    ttyp = htv[t_id]

    # ---- head transform (exact chain on [B,D,2]) ----
    head = ent[h_id]                                          # [B,D,2]
    head = _reflect(head, r1n[htyp], r1_head[htyp])
    rel = r[rel_id]                                           # [B,D,6,2]
    head = _reflect(head, rel[:, :, 0, :], k_head[rel_id])
    for i in range(HOUSD, HOUSE_NUM - HOUSD):
        head = _reflect(head, rel[:, :, i, :])

    # ---- tail transform matrix M[b,d] (2x2): x -> A2 @ A1 @ x ----
    def _refl_mat(rv, k):
        I = np.eye(2)[None, None]
        outer = rv[..., :, None] * rv[..., None, :]
        return I - (2.0 + k)[..., None] * outer

    A1 = _refl_mat(r2n[ttyp], r2_tail[ttyp][:, :, 0:1])
    A2 = _refl_mat(rel[:, :, HOUSE_NUM - 1, :], k_tail[rel_id])
    M = A2 @ A1                                               # [B,D,2,2]

    rwg = rw[rel_id]                                          # [B,D,2]
    Mt = rwg[..., :, None] * M                                # diag(rw) @ M
    a = rwg * head                                            # [B,D,2]

    # ---- Givens QR: Mt = Q T, T upper-triangular ----
    u0, u1 = Mt[..., 0, 0], Mt[..., 0, 1]
    v0, v1 = Mt[..., 1, 0], Mt[..., 1, 1]
    rho = np.sqrt(u0 * u0 + v0 * v0)
    rho_s = np.maximum(rho, 1e-30)
    c, s = u0 / rho_s, v0 / rho_s
    t00 = rho
    t01 = c * u1 + s * v1
    t11 = -s * u1 + c * v1
    a0p = c * a[..., 0] + s * a[..., 1]
    a1p = -s * a[..., 0] + c * a[..., 1]

    cof = np.concatenate([-t00, -t01, -t11, S * a0p, S * a1p],
                         axis=1).astype(np.float32)           # [B,1280]

    # ---- fp8 table: de-interleave to [x0 | x1], scale by S ----
    e32 = np.asarray(inputs["entity_embedding"], np.float32)
    table = (np.concatenate([e32[:, :, 0], e32[:, :, 1]], axis=1)
             * np.float32(S)).astype(F8)                      # [NE,512]
    return table, cof


def emulate(inputs):
    """Numpy emulation of the device numerics for validation."""
    table, cof = precompute(inputs)
    tp = np.asarray(inputs["tail_part"])
    cof = cof.astype(BF16).astype(np.float32)                 # [B,1280]
    bf = lambda z: z.astype(BF16).astype(np.float32)
    rows = table[tp].astype(np.float32)                       # cast fp8->bf16
    x0, x1 = rows[:, :, :256], rows[:, :, 256:]
    T00, T01 = cof[:, None, 0:256], cof[:, None, 256:512]
    T11 = cof[:, None, 512:768]
    A0, A1 = cof[:, None, 768:1024], cof[:, None, 1024:1280]
    w0 = bf(x0 * T00)
    w1 = bf(x1 * T01)
    d0 = bf(bf(w0 + w1) + A0)
    d1 = bf(bf(x1 * T11) + A1)
    e = bf(bf(d0 * d0) + bf(d1 * d1))
    sq = bf(np.sqrt(e))
    r = sq
    for _ in range(3):
        h = r.shape[-1] // 2
        r = bf(r[..., :h] + r[..., h:])
    red = np.sum(r.astype(np.float32), axis=-1)
    return (GAMMA - red / S).astype(np.float32)


# ----------------------------------------------------------------------------
# Device program
# ----------------------------------------------------------------------------
def build_nc5(ne=NE, bc=BC, nt4=NT4, cb=CB):
    import concourse.bacc as bacc
    import concourse.mybir as mybir
    from concourse.bass import IndirectOffsetOnAxis
    from concourse.tile import TileContext

    dt = mybir.dt
    nc = bacc.Bacc("TRN2", target_bir_lowering=False, debug=False,
                   num_devices=NCORES)
    tab = nc.dram_tensor("tab", [ne, 2 * D], dt.float8e4,
                         kind="ExternalInput").ap()
    idx = nc.dram_tensor("idx", [128, bc * nt4], dt.int32,
                         kind="ExternalInput").ap()
    cofs = nc.dram_tensor("cofs", [bc, 5 * D], dt.bfloat16,
                          kind="ExternalInput").ap()
    sel = nc.dram_tensor("sel", [bc, bc * 128], dt.bfloat16,
                         kind="ExternalInput").ap()
    out = nc.dram_tensor("scores", [128, bc * nt4], dt.float32,
                         kind="ExternalOutput").ap()

    mult, add = mybir.AluOpType.mult, mybir.AluOpType.add
    SQRT = mybir.ActivationFunctionType.Sqrt
    SQ = mybir.ActivationFunctionType.Square
    nch = bc // cb
    U = cb * nt4                     # gather tiles per chunk

    def bc4(ap_slice, shape):
        # [128, cb, W] -> [128, cb, nt4, W] with step-0 over nt4
        w = ap_slice.shape[-1]
        return ap_slice.rearrange("p b (o w) -> p b o w", o=1).to_broadcast(
            shape)

    with TileContext(nc) as tc:
        with (
            tc.tile_pool(name="pidx", bufs=1) as pidx,
            tc.tile_pool(name="pcof", bufs=4) as pcof,
            tc.tile_pool(name="px", bufs=6) as px,
            tc.tile_pool(name="pw", bufs=3) as pw,
            tc.tile_pool(name="pe", bufs=4) as pe,
            tc.tile_pool(name="psc", bufs=1) as psc,
            tc.psum_pool(name="pps", bufs=2) as pps,
        ):
            ixt = pidx.tile([128, bc * nt4], dt.int32, tag="ix")
            nc.sync.dma_start(out=ixt[:, 0:U], in_=idx[:, 0:U])
            if U < bc * nt4:
                nc.sync.dma_start(out=ixt[:, U:], in_=idx[:, U:])
            cof_sb = pidx.tile([bc, 5 * D], dt.bfloat16, tag="cofs")
            nc.sync.dma_start(out=cof_sb[:], in_=cofs[:, :])
            sel_sb = pidx.tile([bc, bc * 128], dt.bfloat16, tag="sel")
            nc.sync.dma_start(out=sel_sb[:], in_=sel[:, :])
            score = psc.tile([128, bc * nt4], dt.float32, tag="sc")

            # --- software-pipelined chunk loop ---
            # stage A(c): gathers + cof matmul/copy
            # stage B(c): DVE linear part (W, D1, D0, D+) then ACT Square
            # stage C(c): DVE e-add (emitted 1 chunk late) then ACT Sqrt
            # stage D(c): DVE tree + reduce (emitted 2 chunks late)
            sh2 = [128, cb, nt4, 2 * D]
            sh1 = [128, cb, nt4, D]
            st = {}

            def stage_A(ch):
                ct = pcof.tile([128, cb, 5 * D], dt.bfloat16, tag="cof")
                for j in range(cb):
                    b = ch * cb + j
                    pc = pps.tile([128, 5 * D], dt.float32, tag="pc",
                                  space="PSUM")
                    for c0 in range(0, 5 * D, 512):
                        c1 = min(c0 + 512, 5 * D)
                        nc.tensor.matmul(out=pc[:, c0:c1],
                                         lhsT=sel_sb[:, b * 128:(b + 1) * 128],
                                         rhs=cof_sb[:, c0:c1],
                                         start=True, stop=True)
                    nc.scalar.copy(ct[:, j, :], pc[:])
                X = px.tile([128, U, 2 * D], dt.bfloat16, tag="x")
                for j in range(U):
                    nc.gpsimd.indirect_dma_start(
                        out=X[:, j, :], out_offset=None, in_=tab[:],
                        in_offset=IndirectOffsetOnAxis(
                            ap=ixt[:, ch * U + j:ch * U + j + 1], axis=0))
                st[("x", ch)] = X
                st[("ct", ch)] = ct

            def stage_B(ch):
                X, ct = st.pop(("x", ch)), st.pop(("ct", ch))
                Xv = X[:].rearrange("p (b o) w -> p b o w", b=cb)
                W = pw.tile(sh2, dt.bfloat16, tag="w")
                nc.vector.tensor_tensor(out=W[:], in0=Xv,
                                        in1=bc4(ct[:, :, 0:512], sh2),
                                        op=mult)
                Dt = pw.tile(sh2, dt.bfloat16, tag="d")
                nc.vector.tensor_tensor(out=Dt[:, :, :, 256:512],
                                        in0=Xv[:, :, :, 256:512],
                                        in1=bc4(ct[:, :, 512:768], sh1),
                                        op=mult)
                nc.vector.tensor_tensor(out=Dt[:, :, :, 0:256],
                                        in0=W[:, :, :, 0:256],
                                        in1=W[:, :, :, 256:512], op=add)
                nc.vector.tensor_tensor(out=Dt[:], in0=Dt[:],
                                        in1=bc4(ct[:, :, 768:1280], sh2),
                                        op=add)
                SQt = pw.tile(sh2, dt.bfloat16, tag="sq")
                nc.scalar.activation(SQt[:], Dt[:], SQ)
                st[("sq", ch)] = SQt

            def stage_C(ch):
                SQt = st.pop(("sq", ch))
                E = pe.tile(sh1, dt.bfloat16, tag="e")
                nc.vector.tensor_tensor(out=E[:], in0=SQt[:, :, :, 0:256],
                                        in1=SQt[:, :, :, 256:512], op=add)
                Sq = pe.tile(sh1, dt.bfloat16, tag="s")
                nc.scalar.activation(Sq[:], E[:], SQRT)
                st[("s", ch)] = Sq

            def stage_D(ch):
                Sq = st.pop(("s", ch))
                R1 = pe.tile([128, cb, nt4, D // 2], dt.bfloat16, tag="r1")
                nc.vector.tensor_tensor(out=R1[:], in0=Sq[:, :, :, 0:128],
                                        in1=Sq[:, :, :, 128:256], op=add)
                R2 = pe.tile([128, cb, nt4, D // 4], dt.bfloat16, tag="r2")
                nc.vector.tensor_tensor(out=R2[:], in0=R1[:, :, :, 0:64],
                                        in1=R1[:, :, :, 64:128], op=add)
                R3 = pe.tile([128, cb, nt4, D // 8], dt.bfloat16, tag="r3")
                nc.vector.tensor_tensor(out=R3[:], in0=R2[:, :, :, 0:32],
                                        in1=R2[:, :, :, 32:64], op=add)
                u0 = ch * U
                nc.vector.tensor_reduce(
                    out=score[:, u0:u0 + U].rearrange("p (u o) -> p u o",
                                                      o=1),
                    in_=R3[:], axis=mybir.AxisListType.X,
                    op=add)

            stage_A(0)
            stage_A(1)
            for ch in range(nch):
                if ch + 2 < nch:
                    stage_A(ch + 2)
                stage_B(ch)
                if ch >= 1:
                    stage_C(ch - 1)
                if ch >= 2:
                    stage_D(ch - 2)
            stage_C(nch - 1)
            stage_D(nch - 2)
            stage_D(nch - 1)

            fin = psc.tile([128, bc * nt4], dt.float32, tag="fin")
            nc.vector.tensor_scalar(out=fin[:], in0=score[:],
                                    scalar1=-1.0 / S, scalar2=GAMMA,
                                    op0=mult, op1=add)
            nc.sync.dma_start(out=out[:, :], in_=fin[:])
    nc.compile()
    return nc


def _selmat():
    """[BC, BC*128] bf16 one-hot selectors: block b = ones on row b."""
    s = np.zeros((BC, BC, 128), np.float32)
    s[np.arange(BC), np.arange(BC), :] = 1.0
    return np.ascontiguousarray(s.reshape(BC, BC * 128).astype(BF16))


def _in_maps(inputs):
    table, cof = precompute(inputs)
    tp = np.asarray(inputs["tail_part"]).astype(np.int32)     # [B,NEG]
    cof16 = cof.astype(BF16)
    selmat = _selmat()
    maps = []
    for c in range(NCORES):
        bs = slice(c * BC, (c + 1) * BC)
        ix = tp[bs].reshape(BC, NT4, 128).transpose(2, 0, 1).reshape(
            128, BC * NT4).copy()
        maps.append({
            "tab": table,
            "idx": np.ascontiguousarray(ix),
            "cofs": np.ascontiguousarray(cof16[bs]),
            "sel": selmat,
        })
    return maps


def unscramble(arr, bc=BC, nt4=NT4):
    """[128, bc*nt4] device layout -> [bc, NEG]: scores[b, t*128+p]."""
    return np.ascontiguousarray(
        arr.reshape(128, bc, nt4).transpose(1, 2, 0).reshape(bc, nt4 * 128))


def kernel(**inputs) -> np.ndarray:
    from concourse import bass_utils

    nc = build_nc5()
    res = bass_utils.run_bass_kernel_spmd(
        nc, _in_maps(inputs), core_ids=list(range(NCORES)))
    outs = [unscramble(np.asarray(r["scores"])) for r in res.results]
    return np.concatenate(outs, axis=0).astype(np.float32)


def timed_run(inputs):
    """Traced run for test.py; returns max-core exec time in ns."""
    from concourse import bass_utils

    nc = build_nc5()
    res = bass_utils.run_bass_kernel_spmd(
        nc, _in_maps(inputs), core_ids=list(range(NCORES)), trace=True)
    return res.exec_time_ns


if __name__ == "__main__":
    # quick numpy validation against the reference
    sys.path.insert(0, "/root/problem")
    import os
    os.environ.setdefault("JAX_PLATFORMS", "cpu")
    import reference
    inputs = {k: np.asarray(v) for k, v in reference.setup_inputs().items()}
    exp = np.asarray(reference.reference(**reference.setup_inputs()))
    got = emulate(inputs)
    err = np.abs(got - exp) / np.maximum(np.abs(exp), 1e-6)
    print("emulate rel err: max", err.max(), "mean", err.mean())


# revision 9
# speedup vs baseline: 1.0219x; 1.0038x over previous
"""Trainium2 Bass kernel for nn_KGEModel_57741540327562 (HousE-style KGE scoring).

Strategy (v5):
  - Data-parallel over the batch dim: 8 cores x 32 batch rows each.
  - entity_embedding replicated per core as an fp8(e4m3) table whose rows are
    de-interleaved to [x0(256) | x1(256)] and pre-scaled by S=4096 (so fp8
    only sees well-normalized values; the 1/S is folded into the epilogue).
  - Small relation/type tables folded on the host into per-(b,d) QR
    coefficients (exact float64 math); coefficients pre-broadcast to
    [bc, 128, 1280] bf16 and streamed via HWDGE.
  - Per 2-batch-row chunk: 8 indirect SWDGE gathers (fp8 -> bf16 cast during
    DMA), 5 wide DVE ops for the linear part, ACT Square + Sqrt, a short bf16
    halving tree + fp32 tensor_reduce for the d-sum. TensorE/PSUM unused.

score[b,n] = GAMMA - (1/S) * sum_d sqrt( (Sa0-t00x0'-t01x1')^2 + (Sa1-t11x1')^2 )
where x' = S*x, [t00 t01; 0 t11] = Q^T @ (diag(rw) @ M_tail) per (b,d),
a = Q^T (rw * head).
"""
import sys

sys.path.insert(0, "/opt/trn_rl_repo")

import numpy as np
import ml_dtypes

NE, NR, NT = 200000, 1000, 571
D, HD = 256, 2
HOUSE_NUM, HOUSD = 6, 1
GAMMA, THRED, RTHRED = 10.0, 0.5, 0.8
B, NEG, NCORES = 256, 512, 8
BC = B // NCORES     # batch rows per core
NT4 = NEG // 128     # 128-row gather tiles per batch row
CB = 2               # batch rows per compute chunk
NCH = BC // CB       # chunks per core
S = 4096.0           # fp8 pre-scale
BF16 = ml_dtypes.bfloat16
F8 = ml_dtypes.float8_e4m3


def _l2norm(x, axis=-1):
    n = np.sqrt(np.sum(x * x, axis=axis, keepdims=True))
    return x / np.maximum(n, 1e-12)


def _reflect(x, r, k=0.0):
    c = np.sum(r * x, axis=-1, keepdims=True)
    return x - (2.0 + k) * c * r


def precompute(inputs):
    """Host-side prep: fold small tables into per-(b,d) QR coefficients.

    Returns (table_f8 [NE,512] fp8 scaled by S, cof [B,1280] float32 with
    the a-halves pre-scaled by S): cof row = [-t00 | -t01 | -t11 | Sa0 | Sa1].
    """
    f8t = np.float64
    ent = np.asarray(inputs["entity_embedding"], f8t)         # [NE,D,2]
    rel_emb = np.asarray(inputs["relation_embedding"], f8t)   # [NR,D,12]
    htm = np.asarray(inputs["head_type_mat"], f8t)            # [NT,D,2]
    ttm = np.asarray(inputs["tail_type_mat"], f8t)
    r1_dir = np.asarray(inputs["r1_dir_head"], f8t)           # [NT,1,1]
    r2_dir = np.asarray(inputs["r2_dir_tail"], f8t)
    r1_sc = np.asarray(inputs["r1_scale_head"], f8t)          # [NT,D,1]
    r2_sc = np.asarray(inputs["r2_scale_tail"], f8t)
    k_dir_h = np.asarray(inputs["k_dir_head"], f8t)           # [NR,1,1]
    k_dir_t = np.asarray(inputs["k_dir_tail"], f8t)
    k_sc_h = np.asarray(inputs["k_scale_head"], f8t)          # [NR,D,1]
    k_sc_t = np.asarray(inputs["k_scale_tail"], f8t)
    rw = np.asarray(inputs["relation_weight"], f8t)           # [NR,D,2]
    htv = np.asarray(inputs["head_type_vec"])                 # [NE] int
    hp = np.asarray(inputs["head_part"])                      # [B,3] int

    r = _l2norm(rel_emb.reshape(NR, D, HOUSE_NUM, HD))        # [NR,D,6,2]
    r1n = _l2norm(htm.reshape(NT, D, 1, HD)).reshape(NT, D, HD)
    r2n = _l2norm(ttm.reshape(NT, D, 1, HD)).reshape(NT, D, HD)
    k_head = np.minimum(k_dir_h * np.abs(k_sc_h), THRED)      # [NR,D,1]
    k_tail = np.minimum(k_dir_t * np.abs(k_sc_t), THRED)
    r1_head = np.minimum(r1_dir * np.abs(r1_sc), RTHRED)      # [NT,D,1]
    r2_tail = np.minimum(r2_dir * np.abs(r2_sc), RTHRED)

    h_id, rel_id, t_id = hp[:, 0], hp[:, 1], hp[:, 2]
    htyp = htv[h_id]
    ttyp = htv[t_id]

    # ---- head transform (exact chain on [B,D,2]) ----
    head = ent[h_id]                                          # [B,D,2]
    head = _reflect(head, r1n[htyp], r1_head[htyp])
    rel = r[rel_id]                                           # [B,D,6,2]
    head = _reflect(head, rel[:, :, 0, :], k_head[rel_id])
    for i in range(HOUSD, HOUSE_NUM - HOUSD):
        head = _reflect(head, rel[:, :, i, :])

    # ---- tail transform matrix M[b,d] (2x2): x -> A2 @ A1 @ x ----
    def _refl_mat(rv, k):
        I = np.eye(2)[None, None]
        outer = rv[..., :, None] * rv[..., None, :]
        return I - (2.0 + k)[..., None] * outer

    A1 = _refl_mat(r2n[ttyp], r2_tail[ttyp][:, :, 0:1])
    A2 = _refl_mat(rel[:, :, HOUSE_NUM - 1, :], k_tail[rel_id])
    M = A2 @ A1                                               # [B,D,2,2]

    rwg = rw[rel_id]                                          # [B,D,2]
    Mt = rwg[..., :, None] * M                                # diag(rw) @ M
    a = rwg * head                                            # [B,D,2]

    # ---- Givens QR: Mt = Q T, T upper-triangular ----
    u0, u1 = Mt[..., 0, 0], Mt[..., 0, 1]
    v0, v1 = Mt[..., 1, 0], Mt[..., 1, 1]
    rho = np.sqrt(u0 * u0 + v0 * v0)
    rho_s = np.maximum(rho, 1e-30)
    c, s = u0 / rho_s, v0 / rho_s
    t00 = rho
    t01 = c * u1 + s * v1
    t11 = -s * u1 + c * v1
    a0p = c * a[..., 0] + s * a[..., 1]
    a1p = -s * a[..., 0] + c * a[..., 1]

    cof = np.concatenate([-t00, -t01, -t11, S * a0p, S * a1p],
                         axis=1).astype(np.float32)           # [B,1280]

    # ---- fp8 table: de-interleave to [x0 | x1], scale by S ----
    e32 = np.asarray(inputs["entity_embedding"], np.float32)
    table = (np.concatenate([e32[:, :, 0], e32[:, :, 1]], axis=1)
             * np.float32(S)).astype(F8)                      # [NE,512]
    return table, cof


def emulate(inputs):
    """Numpy emulation of the device numerics for validation."""
    table, cof = precompute(inputs)
    tp = np.asarray(inputs["tail_part"])
    cof = cof.astype(BF16).astype(np.float32)                 # [B,1280]
    bf = lambda z: z.astype(BF16).astype(np.float32)
    rows = table[tp].astype(np.float32)                       # cast fp8->bf16
    x0, x1 = rows[:, :, :256], rows[:, :, 256:]
    T00, T01 = cof[:, None, 0:256], cof[:, None, 256:512]
    T11 = cof[:, None, 512:768]
    A0, A1 = cof[:, None, 768:1024], cof[:, None, 1024:1280]
    w0 = bf(x0 * T00)
    w1 = bf(x1 * T01)
    d0 = bf(bf(w0 + w1) + A0)
    d1 = bf(bf(x1 * T11) + A1)
    e = bf(bf(d0 * d0) + bf(d1 * d1))
    sq = bf(np.sqrt(e))
    r = sq
    for _ in range(3):
        h = r.shape[-1] // 2
        r = bf(r[..., :h] + r[..., h:])
    red = np.sum(r.astype(np.float32), axis=-1)
    return (GAMMA - red / S).astype(np.float32)


# ----------------------------------------------------------------------------
# Device program
# ----------------------------------------------------------------------------
def build_nc5(ne=NE, bc=BC, nt4=NT4, cb=CB):
    import concourse.bacc as bacc
    import concourse.mybir as mybir
    from concourse.bass import IndirectOffsetOnAxis
    from concourse.tile import TileContext

    dt = mybir.dt
    nc = bacc.Bacc("TRN2", target_bir_lowering=False, debug=False,
                   num_devices=NCORES)
    tab = nc.dram_tensor("tab", [ne, 2 * D], dt.float8e4,
                         kind="ExternalInput").ap()
    idx = nc.dram_tensor("idx", [128, bc * nt4], dt.int32,
                         kind="ExternalInput").ap()
    cofs = nc.dram_tensor("cofs", [bc, 5 * D], dt.bfloat16,
                          kind="ExternalInput").ap()
    sel = nc.dram_tensor("sel", [bc, bc * 128], dt.bfloat16,
                         kind="ExternalInput").ap()
    out = nc.dram_tensor("scores", [128, bc * nt4], dt.float32,
                         kind="ExternalOutput").ap()

    mult, add = mybir.AluOpType.mult, mybir.AluOpType.add
    SQRT = mybir.ActivationFunctionType.Sqrt
    SQ = mybir.ActivationFunctionType.Square
    nch = bc // cb
    U = cb * nt4                     # gather tiles per chunk

    def bc4(ap_slice, shape):
        # [128, cb, W] -> [128, cb, nt4, W] with step-0 over nt4
        w = ap_slice.shape[-1]
        return ap_slice.rearrange("p b (o w) -> p b o w", o=1).to_broadcast(
            shape)

    with TileContext(nc) as tc:
        with (
            tc.tile_pool(name="pidx", bufs=1) as pidx,
            tc.tile_pool(name="pcof", bufs=3) as pcof,
            tc.tile_pool(name="px", bufs=8) as px,
            tc.tile_pool(name="pw", bufs=2) as pw,
            tc.tile_pool(name="pe", bufs=3) as pe,
            tc.tile_pool(name="psc", bufs=1) as psc,
            tc.psum_pool(name="pps", bufs=2) as pps,
        ):
            ixt = pidx.tile([128, bc * nt4], dt.int32, tag="ix")
            nc.sync.dma_start(out=ixt[:, 0:U], in_=idx[:, 0:U])
            if U < bc * nt4:
                nc.sync.dma_start(out=ixt[:, U:], in_=idx[:, U:])
            cof_sb = pidx.tile([bc, 5 * D], dt.bfloat16, tag="cofs")
            nc.sync.dma_start(out=cof_sb[:], in_=cofs[:, :])
            sel_sb = pidx.tile([bc, bc * 128], dt.bfloat16, tag="sel")
            nc.sync.dma_start(out=sel_sb[:], in_=sel[:, :])
            score = psc.tile([128, bc * nt4], dt.float32, tag="sc")

            # --- software-pipelined chunk loop ---
            # stage A(c): gathers + cof matmul/copy
            # stage B(c): DVE linear part (W, D1, D0, D+) then ACT Square
            # stage C(c): DVE e-add (emitted 1 chunk late) then ACT Sqrt
            # stage D(c): DVE tree + reduce (emitted 2 chunks late)
            sh2 = [128, cb, nt4, 2 * D]
            sh1 = [128, cb, nt4, D]
            st = {}

            def stage_A(ch):
                ct = pcof.tile([128, cb, 5 * D], dt.bfloat16, tag="cof")
                for j in range(cb):
                    b = ch * cb + j
                    pc = pps.tile([128, 5 * D], dt.float32, tag="pc",
                                  space="PSUM")
                    for c0 in range(0, 5 * D, 512):
                        c1 = min(c0 + 512, 5 * D)
                        nc.tensor.matmul(out=pc[:, c0:c1],
                                         lhsT=sel_sb[:, b * 128:(b + 1) * 128],
                                         rhs=cof_sb[:, c0:c1],
                                         start=True, stop=True)
                    nc.scalar.copy(ct[:, j, :], pc[:])
                X = px.tile([128, U, 2 * D], dt.bfloat16, tag="x")
                for j in range(U):
                    nc.gpsimd.indirect_dma_start(
                        out=X[:, j, :], out_offset=None, in_=tab[:],
                        in_offset=IndirectOffsetOnAxis(
                            ap=ixt[:, ch * U + j:ch * U + j + 1], axis=0))
                st[("x", ch)] = X
                st[("ct", ch)] = ct

            def stage_B(ch):
                X, ct = st.pop(("x", ch)), st.pop(("ct", ch))
                Xv = X[:].rearrange("p (b o) w -> p b o w", b=cb)
                W = pw.tile(sh2, dt.bfloat16, tag="w")
                nc.vector.tensor_tensor(out=W[:], in0=Xv,
                                        in1=bc4(ct[:, :, 0:512], sh2),
                                        op=mult)
                Dt = pw.tile(sh2, dt.bfloat16, tag="d")
                nc.vector.tensor_tensor(out=Dt[:, :, :, 256:512],
                                        in0=Xv[:, :, :, 256:512],
                                        in1=bc4(ct[:, :, 512:768], sh1),
                                        op=mult)
                nc.vector.tensor_tensor(out=Dt[:, :, :, 0:256],
                                        in0=W[:, :, :, 0:256],
                                        in1=W[:, :, :, 256:512], op=add)
                nc.vector.tensor_tensor(out=Dt[:], in0=Dt[:],
                                        in1=bc4(ct[:, :, 768:1280], sh2),
                                        op=add)
                SQt = pw.tile(sh2, dt.bfloat16, tag="sq")
                nc.scalar.activation(SQt[:], Dt[:], SQ)
                st[("sq", ch)] = SQt

            def stage_C(ch):
                SQt = st.pop(("sq", ch))
                E = pe.tile(sh1, dt.bfloat16, tag="e")
                nc.vector.tensor_tensor(out=E[:], in0=SQt[:, :, :, 0:256],
                                        in1=SQt[:, :, :, 256:512], op=add)
                Sq = pe.tile(sh1, dt.bfloat16, tag="s")
                nc.scalar.activation(Sq[:], E[:], SQRT)
                st[("s", ch)] = Sq

            def stage_D(ch):
                Sq = st.pop(("s", ch))
                R1 = pe.tile([128, cb, nt4, D // 2], dt.bfloat16, tag="r1")
                nc.vector.tensor_tensor(out=R1[:], in0=Sq[:, :, :, 0:128],
                                        in1=Sq[:, :, :, 128:256], op=add)
                R2 = pe.tile([128, cb, nt4, D // 4], dt.bfloat16, tag="r2")
                nc.vector.tensor_tensor(out=R2[:], in0=R1[:, :, :, 0:64],
                                        in1=R1[:, :, :, 64:128], op=add)
                R3 = pe.tile([128, cb, nt4, D // 8], dt.bfloat16, tag="r3")
                nc.vector.tensor_tensor(out=R3[:], in0=R2[:, :, :, 0:32],
                                        in1=R2[:, :, :, 32:64], op=add)
                u0 = ch * U
                nc.vector.tensor_reduce(
                    out=score[:, u0:u0 + U].rearrange("p (u o) -> p u o",
                                                      o=1),
                    in_=R3[:], axis=mybir.AxisListType.X,
                    op=add)

            for c0 in range(4):
                stage_A(c0)
            for ch in range(nch):
                if ch + 4 < nch:
                    stage_A(ch + 4)
                stage_B(ch)
                if ch >= 1:
                    stage_C(ch - 1)
                if ch >= 2:
                    stage_D(ch - 2)
            stage_C(nch - 1)
            stage_D(nch - 2)
            stage_D(nch - 1)

            fin = psc.tile([128, bc * nt4], dt.float32, tag="fin")
            nc.vector.tensor_scalar(out=fin[:], in0=score[:],
                                    scalar1=-1.0 / S, scalar2=GAMMA,
                                    op0=mult, op1=add)
            nc.sync.dma_start(out=out[:, :], in_=fin[:])
    nc.compile()
    return nc


def _selmat():
    """[BC, BC*128] bf16 one-hot selectors: block b = ones on row b."""
    s = np.zeros((BC, BC, 128), np.float32)
    s[np.arange(BC), np.arange(BC), :] = 1.0
    return np.ascontiguousarray(s.reshape(BC, BC * 128).astype(BF16))


def _in_maps(inputs):
    table, cof = precompute(inputs)
    tp = np.asarray(inputs["tail_part"]).astype(np.int32)     # [B,NEG]
    cof16 = cof.astype(BF16)
    selmat = _selmat()
    maps = []
    for c in range(NCORES):
        bs = slice(c * BC, (c + 1) * BC)
        ix = tp[bs].reshape(BC, NT4, 128).transpose(2, 0, 1).reshape(
            128, BC * NT4).copy()
        maps.append({
            "tab": table,
            "idx": np.ascontiguousarray(ix),
            "cofs": np.ascontiguousarray(cof16[bs]),
            "sel": selmat,
        })
    return maps


def unscramble(arr, bc=BC, nt4=NT4):
    """[128, bc*nt4] device layout -> [bc, NEG]: scores[b, t*128+p]."""
    return np.ascontiguousarray(
        arr.reshape(128, bc, nt4).transpose(1, 2, 0).reshape(bc, nt4 * 128))


def kernel(**inputs) -> np.ndarray:
    from concourse import bass_utils

    nc = build_nc5()
    res = bass_utils.run_bass_kernel_spmd(
        nc, _in_maps(inputs), core_ids=list(range(NCORES)))
    outs = [unscramble(np.asarray(r["scores"])) for r in res.results]
    return np.concatenate(outs, axis=0).astype(np.float32)


def timed_run(inputs):
    """Traced run for test.py; returns max-core exec time in ns."""
    from concourse import bass_utils

    nc = build_nc5()
    res = bass_utils.run_bass_kernel_spmd(
        nc, _in_maps(inputs), core_ids=list(range(NCORES)), trace=True)
    return res.exec_time_ns


if __name__ == "__main__":
    # quick numpy validation against the reference
    sys.path.insert(0, "/root/problem")
    import os
    os.environ.setdefault("JAX_PLATFORMS", "cpu")
    import reference
    inputs = {k: np.asarray(v) for k, v in reference.setup_inputs().items()}
    exp = np.asarray(reference.reference(**reference.setup_inputs()))
    got = emulate(inputs)
    err = np.abs(got - exp) / np.maximum(np.abs(exp), 1e-6)
    print("emulate rel err: max", err.max(), "mean", err.mean())
